# revision 29
# baseline (speedup 1.0000x reference)
"""Two-layer GATv2 GNN (N=50000, E=800000, 128->4x32->64) on 8 Trainium2
NeuronCores.

Strategy (v4)
-------------
Host: sort raw edges by dst, shard dst nodes contiguously across 8 cores
(6250 each). Per core, dst nodes group into 49 supertiles of 128; each
supertile's incoming NON-self edges pack into 128-edge blocks split into an
A-group (src < 32768) and B-group (src >= 32768) for int16 gather indices
(production gpsimd dma_gather, ~8ns/row). Self-loop edges (one per node) are
handled as one dedicated block per supertile whose "gather" is a contiguous
DMA of the supertile's own feature window.

Device, per layer:
  dense:  sharded node transform (each core computes xl for its 1/8 of
          nodes, rhs = [W_l | W_r] in one N=256 matmul), AllGather xl.
  edges per group (2 supertiles):
    gpsimd dma_gather of xl[src] rows (bf16 256B rows)
    s4  (edge-major dst one-hot)  = is_eq(colix, ek)            on DVE
    s4T (dst-major one-hot)       = is_eq(ones^T @ ekT, rowix)  K=1 outer-
        product matmul broadcasts the per-edge dst slot across partitions
    zr  = s4T_b^T @ xw   (PE, bf16 PSUM)   z = xl_j + zr        on DVE
    logits = att . lrelu(z)  (lrelu + mult on DVE, contiguous h-outer
        reduce), w = exp(logits) on ACT, msg = xl_j * w on DVE
    scatter: PE matmul s4_b^T @ [msg | w] accumulated per supertile into a
        PSUM bank holding both supertiles of the group
  epilogue (dst-node softmax divide, bias, ELU/out) is DEFERRED by one
  group so its PSUM reads never head-of-line-block the DVE queue; the
  divide runs on ACT via per-partition scale = 1/denominator.
Layer-2 node features [xl2 | xr2] pack into one 128-wide bf16 row; one
AllGather shares them.
"""
import numpy as np
import ml_dtypes

import concourse.bass as bass
import concourse.mybir as mybir
from concourse.tile import TileContext
from concourse import library_config
from concourse.bass_utils import run_bass_kernel_spmd

# ---------------- problem constants ----------------
N = 50000
IN = 128
HID = 32
HEADS = 4
H1 = HEADS * HID       # 128
OUT = 64
NCORES = 8
P = 128
PAD_SLOT = 200.0
H_SPLIT = 32768        # int16 gather index split point
ST_G = 2               # supertiles per gather group

F32 = mybir.dt.float32
BF16 = mybir.dt.bfloat16
I16 = mybir.dt.int16
AF = mybir.ActivationFunctionType
ALU = mybir.AluOpType
BF = ml_dtypes.bfloat16


# ------------- walrus workaround -------------
def split_multi_waits(nc):
    """This environment's walrus build rejects any instruction carrying more
    than one sem wait ("Too many sync wait commands"). Move extra waits onto
    engine NOPs inserted immediately before the instruction."""
    import bass_rust
    for f in nc.m.functions:
        for blk in f.blocks:
            il = blk.instructions
            i = 0
            while i < len(il):
                inst = il[i]
                si = inst.sync_info
                if si is not None and si.on_wait is not None and len(si.on_wait) > 1:
                    waits = list(si.on_wait)
                    si.on_wait = waits[-1:]
                    for w in waits[:-1]:
                        nop = nc.engines[inst.engine].nop(nofuse=True).ins
                        cur = nc.cur_bb.bb.instructions
                        assert cur[-1] is nop
                        cur.pop()
                        nop.sync_info = bass_rust.SyncInfo(on_wait=[w], on_update=[])
                        il.insert(i, nop)
                        i += 1
                i += 1


def _wrap16(v):
    """idx k -> [k%16 + 16*r, k//16] for r in 0..7 (replicated across the 8
    gpsimd cores' partition groups)."""
    assert len(v) % 16 == 0
    w = np.zeros((P, len(v) // 16), np.int16)
    t = v.reshape(-1, 16).T
    for r in range(8):
        w[r * 16:(r + 1) * 16, :] = t
    return w


# ---------------- host preprocessing ----------------
def prep(inputs, n=N, ncores=NCORES):
    nloc = n // ncores
    st_n = (nloc + P - 1) // P
    x = np.asarray(inputs["x"], dtype=np.float32)
    ei = np.asarray(inputs["edge_index"])
    W1_l = np.asarray(inputs["W1_l"], np.float32)
    W1_r = np.asarray(inputs["W1_r"], np.float32)
    b1 = np.asarray(inputs["b1"], np.float32)
    att1 = np.asarray(inputs["att1"], np.float32)
    W2_l = np.asarray(inputs["W2_l"], np.float32)
    W2_r = np.asarray(inputs["W2_r"], np.float32)
    b2 = np.asarray(inputs["b2"], np.float32)
    att2 = np.asarray(inputs["att2"], np.float32)

    # raw edges only; self-loops are handled structurally (one per node)
    s_all = ei[0].astype(np.int64)
    d_all = ei[1].astype(np.int64)
    order = np.argsort(d_all, kind="stable")
    s_all = s_all[order]
    d_all = d_all[order]
    bounds = np.searchsorted(d_all, np.arange(ncores + 1) * nloc)

    core_edges = []
    for c in range(ncores):
        lo, hi = bounds[c], bounds[c + 1]
        dl = (d_all[lo:hi] - c * nloc).astype(np.int64)
        sl = s_all[lo:hi]
        core_edges.append((dl, sl))

    # per-supertile A/B block counts (max over cores)
    kA = np.zeros(st_n, np.int64)
    kB = np.zeros(st_n, np.int64)
    per_core_st = []
    for c in range(ncores):
        dl, sl = core_edges[c]
        stc = dl >> 7
        isA = sl < H_SPLIT
        sts = []
        for st in range(st_n):
            m = stc == st
            mA = m & isA
            mB = m & ~isA
            aS, aD = sl[mA], dl[mA]
            bS, bD = sl[mB], dl[mB]
            kA[st] = max(kA[st], (len(aS) + P - 1) // P)
            kB[st] = max(kB[st], (len(bS) + P - 1) // P)
            sts.append((aS, aD, bS, bD))
        per_core_st.append(sts)

    groups = [list(range(g, min(g + ST_G, st_n))) for g in range(0, st_n, ST_G)]
    g_wab = [int(sum(kA[st] + kB[st] for st in sts)) for sts in groups]
    WABmax = max(g_wab)
    BTmax = max(wab + len(sts) for wab, sts in zip(g_wab, groups))

    W1cat = np.concatenate([W1_l, W1_r], axis=1).astype(BF)          # [128,256]
    W2cat = np.concatenate([W2_l, W2_r], axis=1).astype(BF)          # [128,128]
    att1p = np.tile(att1.reshape(1, H1), (P, 1)).astype(BF)          # h-outer
    att2p = np.tile(att2.reshape(1, OUT), (P, 1)).astype(BF)
    b1p = np.tile(b1.reshape(1, H1), (P, 1)).astype(np.float32)
    b2r = np.tile(b2.reshape(1, OUT), (P, 1)).astype(np.float32)
    colix = np.tile(np.arange(P, dtype=np.float32), (P, 1)).astype(BF)
    rowix = np.arange(P, dtype=np.float32).reshape(P, 1).astype(BF)
    ones1 = np.ones((1, P), np.float32).astype(BF)

    n_groups = len(groups)
    in_maps = []
    for c in range(ncores):
        edat = np.zeros((n_groups, P, 8 * WABmax), np.int16)
        ekdat = np.full((n_groups, P, BTmax), PAD_SLOT, BF)
        ektdat = np.full((n_groups, 1, WABmax * P), PAD_SLOT, BF)
        for gi, sts in enumerate(groups):
            WA = int(sum(kA[st] for st in sts))
            WB = int(sum(kB[st] for st in sts))
            WAB = WA + WB
            srcA = np.zeros(WA * P, np.int16)
            srcB = np.zeros(WB * P, np.int16)
            ek = np.full((P, BTmax), PAD_SLOT, np.float32)
            offA = 0
            offB = WA
            for st in sts:
                aS, aD, bS, bD = per_core_st[c][st]
                kk = np.arange(len(aS))
                srcA[offA * P + kk] = aS
                gslot = offA * P + kk
                ek[gslot % P, gslot // P] = aD - (st << 7)
                kk = np.arange(len(bS))
                srcB[(offB - WA) * P + kk] = bS - H_SPLIT
                gslot = offB * P + kk
                ek[gslot % P, gslot // P] = bD - (st << 7)
                offA += int(kA[st])
                offB += int(kB[st])
            # self cols: one per supertile, ek = iota (dst slot == partition)
            for si, st in enumerate(sts):
                cnt = P if st < st_n - 1 else nloc - (st_n - 1) * P
                ek[:cnt, WAB + si] = np.arange(cnt)
            edat[gi, :, :8 * WAB] = np.concatenate(
                [_wrap16(srcA), _wrap16(srcB)], axis=1)
            ekdat[gi] = ek.astype(BF)
            # ekT: dst slot of edge (col b, lane e) laid out along free dim
            ektdat[gi, 0, :WAB * P] = ek[:, :WAB].T.reshape(-1).astype(BF)
        in_maps.append({
            "xTo": np.ascontiguousarray(x[c * nloc:(c + 1) * nloc].T).astype(BF),
            "W1": W1cat, "W2": W2cat,
            "att1p": att1p, "att2p": att2p,
            "b1p": b1p, "b2r": b2r,
            "colix": colix, "rowix": rowix, "ones1": ones1,
            "edat": edat, "ekdat": ekdat, "ektdat": ektdat,
        })
    meta = (kA.tolist(), kB.tolist(), n)
    return in_maps, meta


# ---------------- device program ----------------
DEBUG_DUMP = False
DEBUG_G0 = False


def build_program(meta, reps=1):
    kA, kB, n = meta
    ncores = NCORES
    nloc = n // ncores
    st_n = (nloc + P - 1) // P
    last = nloc - (st_n - 1) * P
    groups = [list(range(g, min(g + ST_G, st_n))) for g in range(0, st_n, ST_G)]
    g_wab = [int(sum(kA[st] + kB[st] for st in sts)) for sts in groups]
    WABmax = max(g_wab)
    BTmax = max(wab + len(sts) for wab, sts in zip(g_wab, groups))
    n_groups = len(groups)
    nb_rows = st_n * P             # padded row count for h

    nc = bass.Bass()
    xTo = nc.dram_tensor("xTo", [P, nloc], BF16, kind="ExternalInput")
    W1 = nc.dram_tensor("W1", [P, 2 * H1], BF16, kind="ExternalInput")
    W2 = nc.dram_tensor("W2", [P, 2 * OUT], BF16, kind="ExternalInput")
    att1p = nc.dram_tensor("att1p", [P, H1], BF16, kind="ExternalInput")
    att2p = nc.dram_tensor("att2p", [P, OUT], BF16, kind="ExternalInput")
    b1p = nc.dram_tensor("b1p", [P, H1], F32, kind="ExternalInput")
    b2r = nc.dram_tensor("b2r", [P, OUT], F32, kind="ExternalInput")
    colix = nc.dram_tensor("colix", [P, P], BF16, kind="ExternalInput")
    rowixt = nc.dram_tensor("rowix", [P, 1], BF16, kind="ExternalInput")
    ones1t = nc.dram_tensor("ones1", [1, P], BF16, kind="ExternalInput")
    edat = nc.dram_tensor("edat", [n_groups, P, 8 * WABmax], I16,
                          kind="ExternalInput")
    ekdat = nc.dram_tensor("ekdat", [n_groups, P, BTmax], BF16,
                           kind="ExternalInput")
    ektdat = nc.dram_tensor("ektdat", [n_groups, 1, WABmax * P], BF16,
                            kind="ExternalInput")
    out_loc = nc.dram_tensor("out_loc", [nloc, OUT], F32, kind="ExternalOutput")

    xl1_loc = nc.dram_tensor("xl1_loc", [nloc, H1], BF16)
    xl1 = nc.dram_tensor("xl1", [n, H1], BF16, addr_space="Shared")
    xr1 = nc.dram_tensor("xr1", [nloc, H1], BF16)
    hdram = nc.dram_tensor("hdram", [nb_rows, H1], BF16)
    t2loc = nc.dram_tensor("t2loc", [nloc, P], BF16)
    t2 = nc.dram_tensor("t2", [n, P], BF16, addr_space="Shared")
    if DEBUG_DUMP:
        dbg_t = {}
        for nm, src in (("xl1_dbg", xl1_loc), ("xr1_dbg", xr1),
                        ("hdram_dbg", hdram), ("t2loc_dbg", t2loc)):
            dbg_t[nm] = (nc.dram_tensor(nm, list(src.shape), BF16,
                                        kind="ExternalOutput"), src)

    cc_sem = nc.alloc_semaphore("cc_sem")

    reg_cache = {}

    def nreg(v):
        if v not in reg_cache:
            reg_cache[v] = nc.gpsimd.to_reg(v)
        return reg_cache[v]

    g0dump = {}

    def g0dma(nm, tile_ap, shape, dtype):
        if nm not in g0dump:
            g0dump[nm] = nc.dram_tensor(nm, list(shape), dtype,
                                        kind="ExternalOutput")
            nc.sync.dma_start(out=g0dump[nm][:], in_=tile_ap)

    def edge_layer(pools, consts, tA, tB, tW, tSelf, dat_w, heads, src_lo,
                   dst_lo, att_sb, bias_sb, layer, dense2=None):
        """3-stage software pipeline: per loop iteration emit
        frontend(g) / backend(g-1) / epilogue(g-2), with epilogue stages
        spread between other ops so no engine queue blocks head-of-line
        on a cross-engine dependency. `dense2(st)` optionally emits the
        layer-2 node transform for a finished supertile (layer 1 only)."""
        pool, psum_agg, psum_zb = pools
        colix_sb = consts["colix"]
        rowix_sb = consts["rowix"]
        ones_sb = consts["ones1"]
        mrow = dat_w + heads
        csub = dat_w // heads

        def frontend(gi, sts):
            WA = int(sum(kA[st] for st in sts))
            WB = int(sum(kB[st] for st in sts))
            WAB = WA + WB
            BT = WAB + len(sts)
            it = pool.tile([P, 8 * WAB], I16, tag="idx")
            nc.sync.dma_start(out=it[:], in_=edat[gi, :, 0:8 * WAB])
            ek = pool.tile([P, BT], BF16, tag="ek")
            nc.sync.dma_start(out=ek[:], in_=ekdat[gi, :, 0:BT])
            ekt = pool.tile([1, WAB * P], BF16, tag="ekt")
            nc.sync.dma_start(out=ekt[:], in_=ektdat[gi, :, 0:WAB * P])
            xzs = pool.tile([P, BT * P], BF16, tag="xzs")
            if WA:
                nc.gpsimd.dma_gather(
                    out_ap=xzs[:, 0:WA * P].rearrange("p (b e) -> p b e", e=P),
                    in_ap=tA, idxs_ap=it[:, 0:8 * WA],
                    num_idxs=WA * P, num_idxs_reg=nreg(WA * P), elem_size=P,
                    single_packet=False)
            if WB:
                nc.gpsimd.dma_gather(
                    out_ap=xzs[:, WA * P:WAB * P]
                        .rearrange("p (b e) -> p b e", e=P),
                    in_ap=tB, idxs_ap=it[:, 8 * WA:8 * WAB],
                    num_idxs=WB * P, num_idxs_reg=nreg(WB * P), elem_size=P,
                    single_packet=False)
            # self blocks: contiguous window loads (instead of gather)
            for si, st in enumerate(sts):
                cnt = P if st < st_n - 1 else last
                sl = xzs[:, (WAB + si) * P:(WAB + si + 1) * P]
                if cnt < P:
                    nc.gpsimd.memset(sl, 0.0)
                nc.scalar.dma_start(
                    out=sl[:cnt, :] if cnt < P else sl,
                    in_=tSelf[st * P:st * P + cnt, :])
            # per-supertile xr windows (contiguous rows, plain DMA)
            xw = {}
            for st in sts:
                cnt = P if st < st_n - 1 else last
                t_xw = pool.tile([P, P], BF16, tag=f"xw{st % ST_G}",
                                 name=f"xw_l{layer}_st{st}")
                if cnt < P:
                    nc.gpsimd.memset(t_xw[:], 0.0)
                nc.sync.dma_start(out=t_xw[:cnt, :],
                                  in_=tW[st * P:st * P + cnt, :])
                xw[st] = t_xw
            blist = []
            for st in sts:
                blist += [st] * int(kA[st])
            for st in sts:
                blist += [st] * int(kB[st])
            # dst-major one-hot s4T via K=1 outer-product broadcast of ekT
            # (emitted before s4 so the PE's zr matmuls unblock earliest)
            s4T = pool.tile([P, WAB * P], BF16, tag="s4T")
            xzs3 = xzs[:].rearrange("p (b e) -> p b e", e=P)
            zt = pool.tile([P, BT * dat_w], BF16, tag="zt")
            zt3 = zt[:].rearrange("p (b f) -> p b f", f=dat_w)
            for c0 in range(0, WAB, 4):
                cw = min(4, WAB - c0)
                pzb = psum_zb.tile([P, 512], F32, tag="pzb",
                                   name=f"pzb_l{layer}_g{gi}_c{c0}")
                nc.tensor.matmul(out=pzb[:, 0:cw * P], lhsT=ones_sb[:],
                                 rhs=ekt[0:1, c0 * P:(c0 + cw) * P],
                                 start=True, stop=True)
                nc.vector.tensor_tensor(
                    out=s4T[:, c0 * P:(c0 + cw) * P],
                    in0=pzb[:, 0:cw * P],
                    in1=rowix_sb[:, 0:1].to_broadcast([P, cw * P]),
                    op=ALU.is_equal)
                # zr overwrites the same bank region (WAR via s4T read)
                for jj in range(cw):
                    b = c0 + jj
                    nc.tensor.matmul(
                        out=pzb[:, jj * dat_w:(jj + 1) * dat_w],
                        lhsT=s4T[:, b * P:(b + 1) * P],
                        rhs=xw[blist[b]][:, dst_lo:dst_lo + dat_w],
                        start=True, stop=True)
                nc.vector.tensor_tensor(
                    out=zt3[:, c0:c0 + cw, :],
                    in0=xzs3[:, c0:c0 + cw, src_lo:src_lo + dat_w],
                    in1=pzb[:, 0:cw * dat_w]
                        .rearrange("p (j f) -> p j f", f=dat_w),
                    op=ALU.add)
            # self blocks: z = xl_i + xr_i directly (s4T/zr not needed)
            for si, st in enumerate(sts):
                nc.vector.tensor_tensor(
                    out=zt3[:, WAB + si, :],
                    in0=xzs3[:, WAB + si, src_lo:src_lo + dat_w],
                    in1=xw[st][:, dst_lo:dst_lo + dat_w],
                    op=ALU.add)
            # edge-major one-hot s4 (for the scatter matmuls of backend)
            s4 = pool.tile([P, BT * P], BF16, tag="s4")
            nc.vector.tensor_tensor(
                out=s4[:].rearrange("p (b q) -> p b q", q=P),
                in0=colix_sb[:, None, :].to_broadcast([P, BT, P]),
                in1=ek[:, :, None].to_broadcast([P, BT, P]),
                op=ALU.is_equal)
            if DEBUG_G0 and layer == 1 and gi == 0:
                g0dma("d_xzs", xzs[:], [P, BT * P], BF16)
                g0dma("d_s4", s4[:], [P, BT * P], BF16)
                g0dma("d_s4T", s4T[:], [P, WAB * P], BF16)
                g0dma("d_zt0", zt[:], [P, BT * dat_w], BF16)
            return dict(gi=gi, sts=sts, WAB=WAB, BT=BT, xzs=xzs, xzs3=xzs3,
                        zt=zt, zt3=zt3, s4=s4, blist=blist)

        def backend_a(f):
            """lrelu + att multiply + logits reduce + exp (DVE/ACT)."""
            gi, sts, BT = f["gi"], f["sts"], f["BT"]
            zt, zt3 = f["zt"], f["zt3"]
            nc.vector.scalar_tensor_tensor(
                out=zt[:], in0=zt[:], scalar=0.2, in1=zt[:],
                op0=ALU.mult, op1=ALU.max)
            nc.vector.tensor_tensor(
                out=zt3, in0=zt3,
                in1=att_sb[:, None, 0:dat_w].to_broadcast([P, BT, dat_w]),
                op=ALU.mult)
            lg = pool.tile([P, BT * heads], F32, tag="lg")
            nc.vector.reduce_sum(
                out=lg[:].rearrange("p (b h) -> p b h", h=heads),
                in_=zt[:].rearrange("p (b h c) -> p b h c",
                                    c=csub, h=heads),
                axis=mybir.AxisListType.X)
            mg = pool.tile([P, BT * mrow], BF16, tag="mg")
            mg3 = mg[:].rearrange("p (b m) -> p b m", m=mrow)
            nc.scalar.activation(
                mg3[:, :, dat_w:dat_w + heads],
                lg[:].rearrange("p (b h) -> p b h", h=heads), AF.Exp)
            f["lg"] = lg
            f["mg"] = mg
            f["mg3"] = mg3

        def backend_b(f):
            """msg multiply (DVE) + per-supertile scatter matmuls (PE)."""
            gi, sts, BT = f["gi"], f["sts"], f["BT"]
            mg, mg3, xzs3, s4 = f["mg"], f["mg3"], f["xzs3"], f["s4"]
            wv = mg3[:, :, dat_w:dat_w + heads, None] \
                .to_broadcast([P, BT, heads, csub])
            nc.vector.tensor_tensor(
                out=mg3[:, :, 0:dat_w].rearrange(
                    "p b (h c) -> p b h c", h=heads),
                in0=xzs3[:, :, src_lo:src_lo + dat_w].rearrange(
                    "p b (h c) -> p b h c", h=heads),
                in1=wv, op=ALU.mult)
            blist_full = f["blist"] + list(sts)
            out = []
            for stl, st in enumerate(sts):
                cols = [t for t, s in enumerate(blist_full) if s == st]
                agg = psum_agg.tile([P, mrow], F32, tag="agg",
                                    name=f"agg_l{layer}_st{st}")
                for i, t in enumerate(cols):
                    nc.tensor.matmul(
                        out=agg[:],
                        lhsT=s4[:, t * P:(t + 1) * P],
                        rhs=mg[:, t * mrow:(t + 1) * mrow],
                        start=(i == 0), stop=(i == len(cols) - 1))
                out.append((st, agg))
                if DEBUG_G0 and layer == 1 and gi == 0:
                    aggc = pool.tile([P, mrow], F32, tag=f"aggdbg{stl}")
                    nc.vector.tensor_copy(out=aggc[:], in_=agg[:])
                    g0dma(f"d_agg{stl}", aggc[:], [P, mrow], F32)
            if DEBUG_G0 and layer == 1 and gi == 0:
                g0dma("d_zt1", f["zt"][:], [P, BT * dat_w], BF16)
                g0dma("d_lg", f["lg"][:], [P, BT * heads], F32)
                g0dma("d_mg", mg[:], [P, BT * mrow], BF16)
            return out

        def epi_s0(e):
            """softmax divide: r0 on DVE, per-head scale on ACT."""
            for k, (st, agg) in enumerate(e):
                r0 = pool.tile([P, heads], F32, tag=f"r0_{k}")
                nc.vector.reciprocal(r0[:], agg[:, dat_w:dat_w + heads])
                ob = pool.tile([P, dat_w], F32, tag=f"ob_{k}")
                for h in range(heads):
                    nc.scalar.activation(
                        ob[:, h * csub:(h + 1) * csub],
                        agg[:, h * csub:(h + 1) * csub],
                        AF.Copy, scale=r0[:, h:h + 1])
                e[k] = (st, ob)

        def epi_s1(e):
            """bias add (DVE, SBUF-only)."""
            for k, (st, ob) in enumerate(e):
                ob2 = pool.tile([P, dat_w], F32, tag=f"ob2_{k}")
                nc.vector.tensor_tensor(
                    out=ob2[:], in0=ob[:], in1=bias_sb[:], op=ALU.add)
                e[k] = (st, ob2)

        def epi_s2(e):
            """ELU pieces on ACT: t2=exp(min(x,0)) via relu(-x), t3=relu(x)."""
            if layer != 1:
                return
            for k, (st, ob2) in enumerate(e):
                t1 = pool.tile([P, dat_w], F32, tag=f"t1_{k}")
                nc.scalar.activation(t1[:], ob2[:], AF.Relu, scale=-1.0)
                t2 = pool.tile([P, dat_w], F32, tag=f"t2_{k}")
                nc.scalar.activation(t2[:], t1[:], AF.Exp, scale=-1.0)
                t3 = pool.tile([P, dat_w], F32, tag=f"t3_{k}")
                nc.scalar.activation(t3[:], ob2[:], AF.Relu)
                e[k] = (st, (t2, t3))

        def epi_s3(e):
            """combine + store (+ interleaved dense-2 for layer 1)."""
            for k, (st, v) in enumerate(e):
                cnt = P if st < st_n - 1 else last
                if layer == 1:
                    t2, t3 = v
                    he = pool.tile([P, dat_w], BF16, tag=f"he_{k}")
                    nc.vector.scalar_tensor_tensor(
                        out=he[:], in0=t2[:], scalar=-1.0, in1=t3[:],
                        op0=ALU.add, op1=ALU.add)
                    if DEBUG_DUMP:
                        nc.sync.dma_start(
                            out=hdram[st * P:(st + 1) * P, :], in_=he[:])
                    if dense2 is not None:
                        dense2(st, he)
                else:
                    nc.sync.dma_start(
                        out=out_loc[st * P:st * P + cnt, :],
                        in_=v[:cnt, :])

        fprev = None   # frontend output of group g-1
        eprev = None   # epilogue state of group g-2
        for i in range(len(groups) + 2):
            f = frontend(i, groups[i]) if i < len(groups) else None
            if eprev is not None:
                epi_s0(eprev)
            if fprev is not None:
                backend_a(fprev)
            if eprev is not None:
                epi_s1(eprev)
                epi_s2(eprev)
            aggs = backend_b(fprev) if fprev is not None else None
            if eprev is not None:
                epi_s3(eprev)
            eprev = aggs
            fprev = f

    for rep in range(reps):
        # ---- TC_A: sharded dense-1 (xl1 + xr1 for local nodes) ----
        with TileContext(nc) as tc:
            with tc.tile_pool(name="constA", bufs=1) as cpool, \
                 tc.tile_pool(name="dA", bufs=4) as dpool, \
                 tc.tile_pool(name="pdA", bufs=2, space="PSUM") as psum_d:
                w1_sb = cpool.tile([P, 2 * H1], BF16)
                nc.sync.dma_start(out=w1_sb[:], in_=W1[:])
                for t in range(st_n):
                    cols = P if t < st_n - 1 else last
                    xt = dpool.tile([P, P], BF16, tag="xt")
                    nc.sync.dma_start(out=xt[:, :cols],
                                      in_=xTo[:, t * P:t * P + cols])
                    psd = psum_d.tile([cols, 2 * H1], F32, tag="psd")
                    nc.tensor.matmul(out=psd[:], lhsT=xt[:, :cols],
                                     rhs=w1_sb[:], start=True, stop=True)
                    sb = dpool.tile([cols, 2 * H1], BF16, tag="sbd")
                    nc.scalar.copy(out=sb[:, 0:H1], in_=psd[:, 0:H1])
                    nc.vector.tensor_copy(out=sb[:, H1:2 * H1],
                                          in_=psd[:, H1:2 * H1])
                    nc.sync.dma_start(out=xl1_loc[t * P:t * P + cols, :],
                                      in_=sb[:, 0:H1])
                    nc.sync.dma_start(out=xr1[t * P:t * P + cols, :],
                                      in_=sb[:, H1:2 * H1])

        # ---- AllGather xl1_loc -> xl1 ----
        nc.gpsimd.collective_compute(
            "AllGather", ALU.bypass,
            replica_groups=[list(range(ncores))],
            ins=[xl1_loc[:]], outs=[xl1[:]],
        ).then_inc(cc_sem)
        nc.gpsimd.wait_ge(cc_sem, 2 * rep + 1)

        # ---- TC_B: L1 edges + interleaved dense-2 ----
        with TileContext(nc) as tc:
            with tc.tile_pool(name="const", bufs=1) as cpool, \
                 tc.tile_pool(name="work", bufs=2) as pool, \
                 tc.tile_pool(name="pagg", bufs=4, space="PSUM") as psum_agg, \
                 tc.tile_pool(name="pzb", bufs=2, space="PSUM") as psum_zb, \
                 tc.tile_pool(name="paux", bufs=2, space="PSUM") as psum_aux:
                nc.gpsimd.load_library(library_config.mlp)
                att1_sb = cpool.tile([P, H1], BF16)
                nc.sync.dma_start(out=att1_sb[:], in_=att1p[:])
                b1_sb = cpool.tile([P, H1], F32)
                nc.sync.dma_start(out=b1_sb[:], in_=b1p[:])
                colix_sb = cpool.tile([P, P], BF16)
                nc.sync.dma_start(out=colix_sb[:], in_=colix[:])
                rowix_sb = cpool.tile([P, 1], BF16)
                nc.sync.dma_start(out=rowix_sb[:], in_=rowixt[:])
                ones_sb = cpool.tile([1, P], BF16)
                nc.sync.dma_start(out=ones_sb[:], in_=ones1t[:])
                w2_sb = cpool.tile([P, 2 * OUT], BF16)
                nc.sync.dma_start(out=w2_sb[:], in_=W2[:])
                consts = {"colix": colix_sb, "rowix": rowix_sb,
                          "ones1": ones_sb}

                def dense2(st, he):
                    cols = P if st < st_n - 1 else last
                    xh = pool.tile([P, P], BF16, tag="xh")
                    nc.sync.dma_start_transpose(out=xh[:], in_=he[:])
                    psd2 = psum_aux.tile([P, 2 * OUT], F32, tag="psd2",
                                         name=f"psd2_{st}")
                    nc.tensor.matmul(out=psd2[:cols, :], lhsT=xh[:, :cols],
                                     rhs=w2_sb[:], start=True, stop=True)
                    sb2 = pool.tile([P, 2 * OUT], BF16, tag="sb2")
                    nc.scalar.copy(out=sb2[:cols, :], in_=psd2[:cols, :])
                    nc.sync.dma_start(out=t2loc[st * P:st * P + cols, :],
                                      in_=sb2[:cols, :])

                edge_layer((pool, psum_agg, psum_zb), consts,
                           xl1[0:H_SPLIT, :], xl1[H_SPLIT:n, :], xr1,
                           xl1_loc,
                           H1, HEADS, 0, 0, att1_sb, b1_sb, layer=1,
                           dense2=dense2)

        if DEBUG_DUMP and rep == 0:
            with TileContext(nc):
                for nm, (dst, src) in dbg_t.items():
                    nc.sync.dma_start(out=dst[:], in_=src[:])

        # ---- AllGather t2loc -> t2 ----
        nc.gpsimd.collective_compute(
            "AllGather", ALU.bypass,
            replica_groups=[list(range(ncores))],
            ins=[t2loc[:]], outs=[t2[:]],
        ).then_inc(cc_sem)
        nc.gpsimd.wait_ge(cc_sem, 2 * rep + 2)

        # ---- TC_C: edges layer 2 ----
        with TileContext(nc) as tc:
            with tc.tile_pool(name="const2", bufs=1) as cpool, \
                 tc.tile_pool(name="work2", bufs=2) as pool, \
                 tc.tile_pool(name="pagg2", bufs=4, space="PSUM") as psum_agg, \
                 tc.tile_pool(name="pzb2", bufs=2, space="PSUM") as psum_zb:
                nc.gpsimd.load_library(library_config.mlp)
                att2_sb = cpool.tile([P, OUT], BF16)
                nc.sync.dma_start(out=att2_sb[:], in_=att2p[:])
                b2_sb = cpool.tile([P, OUT], F32)
                nc.sync.dma_start(out=b2_sb[:], in_=b2r[:])
                colix_sb = cpool.tile([P, P], BF16)
                nc.sync.dma_start(out=colix_sb[:], in_=colix[:])
                rowix_sb = cpool.tile([P, 1], BF16)
                nc.sync.dma_start(out=rowix_sb[:], in_=rowixt[:])
                ones_sb = cpool.tile([1, P], BF16)
                nc.sync.dma_start(out=ones_sb[:], in_=ones1t[:])
                consts = {"colix": colix_sb, "rowix": rowix_sb,
                          "ones1": ones_sb}
                edge_layer((pool, psum_agg, psum_zb), consts,
                           t2[0:H_SPLIT, :], t2[H_SPLIT:n, :], t2loc,
                           t2loc,
                           OUT, 1, 0, OUT, att2_sb, b2_sb, layer=2)

    split_multi_waits(nc)
    mybir.codegen_inst_isa_subclasses(nc)
    return nc


# ---------------- entry point ----------------
def kernel(**inputs) -> np.ndarray:
    in_maps, meta = prep(inputs)
    nc = build_program(meta)
    res = run_bass_kernel_spmd(nc, in_maps, list(range(NCORES)))
    out = np.concatenate([res.results[c]["out_loc"] for c in range(NCORES)],
                         axis=0)
    return out.astype(np.float32)


# revision 40
# speedup vs baseline: 1.3811x; 1.3811x over previous
"""Two-layer GATv2 GNN (N=50000, E=800000, 128->4x32->64) on 8 Trainium2
NeuronCores.

Strategy (v4)
-------------
Host: sort raw edges by dst, shard dst nodes contiguously across 8 cores
(6250 each). Per core, dst nodes group into 49 supertiles of 128; each
supertile's incoming NON-self edges pack into 128-edge blocks split into an
A-group (src < 32768) and B-group (src >= 32768) for int16 gather indices
(production gpsimd dma_gather, ~8ns/row). Self-loop edges (one per node) are
handled as one dedicated block per supertile whose "gather" is a contiguous
DMA of the supertile's own feature window.

Device, per layer:
  dense:  sharded node transform (each core computes xl for its 1/8 of
          nodes, rhs = [W_l | W_r] in one N=256 matmul), AllGather xl.
  edges per group (2 supertiles):
    gpsimd dma_gather of xl[src] rows (bf16 256B rows)
    s4  (edge-major dst one-hot)  = is_eq(colix, ek)            on DVE
    s4T (dst-major one-hot)       = is_eq(ones^T @ ekT, rowix)  K=1 outer-
        product matmul broadcasts the per-edge dst slot across partitions
    zr  = s4T_b^T @ xw   (PE, bf16 PSUM)   z = xl_j + zr        on DVE
    logits = att . lrelu(z)  (lrelu + mult on DVE, contiguous h-outer
        reduce), w = exp(logits) on ACT, msg = xl_j * w on DVE
    scatter: PE matmul s4_b^T @ [msg | w] accumulated per supertile into a
        PSUM bank holding both supertiles of the group
  epilogue (dst-node softmax divide, bias, ELU/out) is DEFERRED by one
  group so its PSUM reads never head-of-line-block the DVE queue; the
  divide runs on ACT via per-partition scale = 1/denominator.
Layer-2 node features [xl2 | xr2] pack into one 128-wide bf16 row; one
AllGather shares them.
"""
import numpy as np
import ml_dtypes

import concourse.bass as bass
import concourse.mybir as mybir
from concourse.tile import TileContext
from concourse import library_config
from concourse.bass_utils import run_bass_kernel_spmd

# ---------------- problem constants ----------------
N = 50000
IN = 128
HID = 32
HEADS = 4
H1 = HEADS * HID       # 128
OUT = 64
NCORES = 8
P = 128
PAD_SLOT = 200.0
H_SPLIT = 32768        # int16 gather index split point
ST_G = 2               # supertiles per gather group

F32 = mybir.dt.float32
BF16 = mybir.dt.bfloat16
I16 = mybir.dt.int16
AF = mybir.ActivationFunctionType
ALU = mybir.AluOpType
BF = ml_dtypes.bfloat16


# ------------- walrus workaround -------------
def split_multi_waits(nc):
    """This environment's walrus build rejects any instruction carrying more
    than one sem wait ("Too many sync wait commands"). Move extra waits onto
    engine NOPs inserted immediately before the instruction."""
    import bass_rust
    for f in nc.m.functions:
        for blk in f.blocks:
            il = blk.instructions
            i = 0
            while i < len(il):
                inst = il[i]
                si = inst.sync_info
                if si is not None and si.on_wait is not None and len(si.on_wait) > 1:
                    waits = list(si.on_wait)
                    si.on_wait = waits[-1:]
                    for w in waits[:-1]:
                        nop = nc.engines[inst.engine].nop(nofuse=True).ins
                        cur = nc.cur_bb.bb.instructions
                        assert cur[-1] is nop
                        cur.pop()
                        nop.sync_info = bass_rust.SyncInfo(on_wait=[w], on_update=[])
                        il.insert(i, nop)
                        i += 1
                i += 1


def _wrap16(v):
    """idx k -> [k%16 + 16*r, k//16] for r in 0..7 (replicated across the 8
    gpsimd cores' partition groups)."""
    assert len(v) % 16 == 0
    w = np.zeros((P, len(v) // 16), np.int16)
    t = v.reshape(-1, 16).T
    for r in range(8):
        w[r * 16:(r + 1) * 16, :] = t
    return w


# ---------------- host preprocessing ----------------
def prep(inputs, n=N, ncores=NCORES):
    nloc = n // ncores
    st_n = (nloc + P - 1) // P
    x = np.asarray(inputs["x"], dtype=np.float32)
    ei = np.asarray(inputs["edge_index"])
    W1_l = np.asarray(inputs["W1_l"], np.float32)
    W1_r = np.asarray(inputs["W1_r"], np.float32)
    b1 = np.asarray(inputs["b1"], np.float32)
    att1 = np.asarray(inputs["att1"], np.float32)
    W2_l = np.asarray(inputs["W2_l"], np.float32)
    W2_r = np.asarray(inputs["W2_r"], np.float32)
    b2 = np.asarray(inputs["b2"], np.float32)
    att2 = np.asarray(inputs["att2"], np.float32)

    # raw edges only; self-loops are handled structurally (one per node)
    s_all = ei[0].astype(np.int64)
    d_all = ei[1].astype(np.int64)
    order = np.argsort(d_all, kind="stable")
    s_all = s_all[order]
    d_all = d_all[order]
    bounds = np.searchsorted(d_all, np.arange(ncores + 1) * nloc)

    core_edges = []
    for c in range(ncores):
        lo, hi = bounds[c], bounds[c + 1]
        dl = (d_all[lo:hi] - c * nloc).astype(np.int64)
        sl = s_all[lo:hi]
        core_edges.append((dl, sl))

    # per-supertile A/B block counts (max over cores)
    kA = np.zeros(st_n, np.int64)
    kB = np.zeros(st_n, np.int64)
    per_core_st = []
    for c in range(ncores):
        dl, sl = core_edges[c]
        stc = dl >> 7
        isA = sl < H_SPLIT
        sts = []
        for st in range(st_n):
            m = stc == st
            mA = m & isA
            mB = m & ~isA
            aS, aD = sl[mA], dl[mA]
            bS, bD = sl[mB], dl[mB]
            kA[st] = max(kA[st], (len(aS) + P - 1) // P)
            kB[st] = max(kB[st], (len(bS) + P - 1) // P)
            sts.append((aS, aD, bS, bD))
        per_core_st.append(sts)

    groups = [list(range(g, min(g + ST_G, st_n))) for g in range(0, st_n, ST_G)]
    g_wab = [int(sum(kA[st] + kB[st] for st in sts)) for sts in groups]
    WABmax = max(g_wab)
    BTmax = max(wab + len(sts) for wab, sts in zip(g_wab, groups))

    W1cat = np.concatenate([W1_l, W1_r], axis=1).astype(BF)          # [128,256]
    W2cat = np.concatenate([W2_l, W2_r], axis=1).astype(BF)          # [128,128]
    att1p = np.tile(att1.reshape(1, H1), (P, 1)).astype(BF)          # h-outer
    att2p = np.tile(att2.reshape(1, OUT), (P, 1)).astype(BF)
    b1p = np.tile(b1.reshape(1, H1), (P, 1)).astype(np.float32)
    b2r = np.tile(b2.reshape(1, OUT), (P, 1)).astype(np.float32)
    colix = np.tile(np.arange(P, dtype=np.float32), (P, 1)).astype(BF)
    rowix = np.arange(P, dtype=np.float32).reshape(P, 1).astype(BF)
    ones1 = np.ones((1, P), np.float32).astype(BF)

    n_groups = len(groups)
    in_maps = []
    for c in range(ncores):
        edat = np.zeros((n_groups, P, 8 * WABmax), np.int16)
        ekdat = np.full((n_groups, P, BTmax), PAD_SLOT, BF)
        ektdat = np.full((n_groups, 1, WABmax * P), PAD_SLOT, BF)
        for gi, sts in enumerate(groups):
            WA = int(sum(kA[st] for st in sts))
            WB = int(sum(kB[st] for st in sts))
            WAB = WA + WB
            srcA = np.zeros(WA * P, np.int16)
            srcB = np.zeros(WB * P, np.int16)
            ek = np.full((P, BTmax), PAD_SLOT, np.float32)
            offA = 0
            offB = WA
            for st in sts:
                aS, aD, bS, bD = per_core_st[c][st]
                kk = np.arange(len(aS))
                srcA[offA * P + kk] = aS
                gslot = offA * P + kk
                ek[gslot % P, gslot // P] = aD - (st << 7)
                kk = np.arange(len(bS))
                srcB[(offB - WA) * P + kk] = bS - H_SPLIT
                gslot = offB * P + kk
                ek[gslot % P, gslot // P] = bD - (st << 7)
                offA += int(kA[st])
                offB += int(kB[st])
            # self cols: one per supertile, ek = iota (dst slot == partition)
            for si, st in enumerate(sts):
                cnt = P if st < st_n - 1 else nloc - (st_n - 1) * P
                ek[:cnt, WAB + si] = np.arange(cnt)
            edat[gi, :, :8 * WAB] = np.concatenate(
                [_wrap16(srcA), _wrap16(srcB)], axis=1)
            ekdat[gi] = ek.astype(BF)
            # ekT: dst slot of edge (col b, lane e) laid out along free dim
            ektdat[gi, 0, :WAB * P] = ek[:, :WAB].T.reshape(-1).astype(BF)
        in_maps.append({
            "xTo": np.ascontiguousarray(x[c * nloc:(c + 1) * nloc].T).astype(BF),
            "W1": W1cat, "W2": W2cat,
            "att1p": att1p, "att2p": att2p,
            "b1p": b1p, "b2r": b2r,
            "colix": colix, "rowix": rowix, "ones1": ones1,
            "edat": edat, "ekdat": ekdat, "ektdat": ektdat,
        })
    meta = (kA.tolist(), kB.tolist(), n)
    return in_maps, meta


# ---------------- device program ----------------
DEBUG_DUMP = False
DEBUG_G0 = False
PIPE_SKEW = 1   # 1: backend lags frontend by 1 group; 2: by 2 groups


def build_program(meta, reps=1):
    kA, kB, n = meta
    ncores = NCORES
    nloc = n // ncores
    st_n = (nloc + P - 1) // P
    last = nloc - (st_n - 1) * P
    groups = [list(range(g, min(g + ST_G, st_n))) for g in range(0, st_n, ST_G)]
    g_wab = [int(sum(kA[st] + kB[st] for st in sts)) for sts in groups]
    WABmax = max(g_wab)
    BTmax = max(wab + len(sts) for wab, sts in zip(g_wab, groups))
    n_groups = len(groups)
    nb_rows = st_n * P             # padded row count for h

    nc = bass.Bass()
    xTo = nc.dram_tensor("xTo", [P, nloc], BF16, kind="ExternalInput")
    W1 = nc.dram_tensor("W1", [P, 2 * H1], BF16, kind="ExternalInput")
    W2 = nc.dram_tensor("W2", [P, 2 * OUT], BF16, kind="ExternalInput")
    att1p = nc.dram_tensor("att1p", [P, H1], BF16, kind="ExternalInput")
    att2p = nc.dram_tensor("att2p", [P, OUT], BF16, kind="ExternalInput")
    b1p = nc.dram_tensor("b1p", [P, H1], F32, kind="ExternalInput")
    b2r = nc.dram_tensor("b2r", [P, OUT], F32, kind="ExternalInput")
    colix = nc.dram_tensor("colix", [P, P], BF16, kind="ExternalInput")
    rowixt = nc.dram_tensor("rowix", [P, 1], BF16, kind="ExternalInput")
    ones1t = nc.dram_tensor("ones1", [1, P], BF16, kind="ExternalInput")
    edat = nc.dram_tensor("edat", [n_groups, P, 8 * WABmax], I16,
                          kind="ExternalInput")
    ekdat = nc.dram_tensor("ekdat", [n_groups, P, BTmax], BF16,
                           kind="ExternalInput")
    ektdat = nc.dram_tensor("ektdat", [n_groups, 1, WABmax * P], BF16,
                            kind="ExternalInput")
    out_loc = nc.dram_tensor("out_loc", [nloc, OUT], F32, kind="ExternalOutput")

    xl1_loc = nc.dram_tensor("xl1_loc", [nloc, H1], BF16)
    xl1 = nc.dram_tensor("xl1", [n, H1], BF16, addr_space="Shared")
    xr1 = nc.dram_tensor("xr1", [nloc, H1], BF16)
    hdram = nc.dram_tensor("hdram", [nb_rows, H1], BF16)
    t2loc = nc.dram_tensor("t2loc", [nloc, P], BF16)
    t2 = nc.dram_tensor("t2", [n, P], BF16, addr_space="Shared")
    if DEBUG_DUMP:
        dbg_t = {}
        for nm, src in (("xl1_dbg", xl1_loc), ("xr1_dbg", xr1),
                        ("hdram_dbg", hdram), ("t2loc_dbg", t2loc)):
            dbg_t[nm] = (nc.dram_tensor(nm, list(src.shape), BF16,
                                        kind="ExternalOutput"), src)

    cc_sem = nc.alloc_semaphore("cc_sem")

    reg_cache = {}

    def nreg(v):
        if v not in reg_cache:
            reg_cache[v] = nc.gpsimd.to_reg(v)
        return reg_cache[v]

    g0dump = {}

    def g0dma(nm, tile_ap, shape, dtype):
        if nm not in g0dump:
            g0dump[nm] = nc.dram_tensor(nm, list(shape), dtype,
                                        kind="ExternalOutput")
            nc.sync.dma_start(out=g0dump[nm][:], in_=tile_ap)

    def edge_layer(pools, consts, tA, tB, tW, tSelf, dat_w, heads, src_lo,
                   dst_lo, att_sb, bias_sb, layer, dense2=None):
        """3-stage software pipeline: per loop iteration emit
        frontend(g) / backend(g-1) / epilogue(g-2), with epilogue stages
        spread between other ops so no engine queue blocks head-of-line
        on a cross-engine dependency. `dense2(st)` optionally emits the
        layer-2 node transform for a finished supertile (layer 1 only)."""
        pool, psum_agg, psum_zb = pools
        colix_sb = consts["colix"]
        rowix_sb = consts["rowix"]
        ones_sb = consts["ones1"]
        mrow = dat_w + heads
        csub = dat_w // heads

        def frontend_loads(gi, sts):
            WA = int(sum(kA[st] for st in sts))
            WB = int(sum(kB[st] for st in sts))
            WAB = WA + WB
            BT = WAB + len(sts)
            it = pool.tile([P, 8 * WAB], I16, tag="idx")
            nc.sync.dma_start(out=it[:], in_=edat[gi, :, 0:8 * WAB])
            ek = pool.tile([P, BT], BF16, tag="ek")
            nc.sync.dma_start(out=ek[:], in_=ekdat[gi, :, 0:BT])
            ekt = pool.tile([1, WAB * P], BF16, tag="ekt")
            nc.sync.dma_start(out=ekt[:], in_=ektdat[gi, :, 0:WAB * P])
            xzs = pool.tile([P, BT * P], BF16, tag="xzs", bufs=3)
            if WA:
                nc.gpsimd.dma_gather(
                    out_ap=xzs[:, 0:WA * P].rearrange("p (b e) -> p b e", e=P),
                    in_ap=tA, idxs_ap=it[:, 0:8 * WA],
                    num_idxs=WA * P, num_idxs_reg=nreg(WA * P), elem_size=P,
                    single_packet=False)
            if WB:
                nc.gpsimd.dma_gather(
                    out_ap=xzs[:, WA * P:WAB * P]
                        .rearrange("p (b e) -> p b e", e=P),
                    in_ap=tB, idxs_ap=it[:, 8 * WA:8 * WAB],
                    num_idxs=WB * P, num_idxs_reg=nreg(WB * P), elem_size=P,
                    single_packet=False)
            # self blocks: contiguous window loads (instead of gather)
            for si, st in enumerate(sts):
                cnt = P if st < st_n - 1 else last
                sl = xzs[:, (WAB + si) * P:(WAB + si + 1) * P]
                if cnt < P:
                    nc.gpsimd.memset(sl, 0.0)
                nc.scalar.dma_start(
                    out=sl[:cnt, :] if cnt < P else sl,
                    in_=tSelf[st * P:st * P + cnt, :])
            # per-supertile xr windows (contiguous rows, plain DMA)
            xw = {}
            for st in sts:
                cnt = P if st < st_n - 1 else last
                t_xw = pool.tile([P, P], BF16, tag=f"xw{st % ST_G}",
                                 name=f"xw_l{layer}_st{st}")
                if cnt < P:
                    nc.gpsimd.memset(t_xw[:], 0.0)
                nc.sync.dma_start(out=t_xw[:cnt, :],
                                  in_=tW[st * P:st * P + cnt, :])
                xw[st] = t_xw
            blist = []
            for st in sts:
                blist += [st] * int(kA[st])
            for st in sts:
                blist += [st] * int(kB[st])
            return dict(gi=gi, sts=sts, WA=WA, WB=WB, WAB=WAB, BT=BT,
                        it=it, ek=ek, ekt=ekt, xzs=xzs, xw=xw, blist=blist)

        def frontend_compute(f):
            gi, sts = f["gi"], f["sts"]
            WAB, BT = f["WAB"], f["BT"]
            ek, ekt, xzs, xw, blist = \
                f["ek"], f["ekt"], f["xzs"], f["xw"], f["blist"]
            # dst-major one-hot s4T via K=1 outer-product broadcast of ekT
            s4T = pool.tile([P, WAB * P], BF16, tag="s4T")
            xzs3 = xzs[:].rearrange("p (b e) -> p b e", e=P)
            zt = pool.tile([P, BT * dat_w], BF16, tag="zt", bufs=3)
            zt3 = zt[:].rearrange("p (b f) -> p b f", f=dat_w)
            for c0 in range(0, WAB, 4):
                cw = min(4, WAB - c0)
                pzb = psum_zb.tile([P, 512], F32, tag="pzb",
                                   name=f"pzb_l{layer}_g{gi}_c{c0}")
                nc.tensor.matmul(out=pzb[:, 0:cw * P], lhsT=ones_sb[:],
                                 rhs=ekt[0:1, c0 * P:(c0 + cw) * P],
                                 start=True, stop=True)
                nc.vector.tensor_tensor(
                    out=s4T[:, c0 * P:(c0 + cw) * P],
                    in0=pzb[:, 0:cw * P],
                    in1=rowix_sb[:, 0:1].to_broadcast([P, cw * P]),
                    op=ALU.is_equal)
                # zr overwrites the same bank region (WAR via s4T read)
                for jj in range(cw):
                    b = c0 + jj
                    nc.tensor.matmul(
                        out=pzb[:, jj * dat_w:(jj + 1) * dat_w],
                        lhsT=s4T[:, b * P:(b + 1) * P],
                        rhs=xw[blist[b]][:, dst_lo:dst_lo + dat_w],
                        start=True, stop=True)
                nc.vector.tensor_tensor(
                    out=zt3[:, c0:c0 + cw, :],
                    in0=xzs3[:, c0:c0 + cw, src_lo:src_lo + dat_w],
                    in1=pzb[:, 0:cw * dat_w]
                        .rearrange("p (j f) -> p j f", f=dat_w),
                    op=ALU.add)
            # self blocks: z = xl_i + xr_i directly (s4T/zr not needed)
            for si, st in enumerate(sts):
                nc.vector.tensor_tensor(
                    out=zt3[:, WAB + si, :],
                    in0=xzs3[:, WAB + si, src_lo:src_lo + dat_w],
                    in1=xw[st][:, dst_lo:dst_lo + dat_w],
                    op=ALU.add)
            # edge-major one-hot s4 (for the scatter matmuls of backend)
            s4 = pool.tile([P, BT * P], BF16, tag="s4", bufs=3)
            nc.vector.tensor_tensor(
                out=s4[:].rearrange("p (b q) -> p b q", q=P),
                in0=colix_sb[:, None, :].to_broadcast([P, BT, P]),
                in1=ek[:, :, None].to_broadcast([P, BT, P]),
                op=ALU.is_equal)
            if DEBUG_G0 and layer == 1 and gi == 0:
                g0dma("d_xzs", xzs[:], [P, BT * P], BF16)
                g0dma("d_s4", s4[:], [P, BT * P], BF16)
                g0dma("d_s4T", s4T[:], [P, WAB * P], BF16)
                g0dma("d_zt0", zt[:], [P, BT * dat_w], BF16)
            f["xzs3"] = xzs3
            f["zt"] = zt
            f["zt3"] = zt3
            f["s4"] = s4

        def backend_a(f):
            """lrelu + att multiply + logits reduce + exp (DVE/ACT)."""
            gi, sts, BT = f["gi"], f["sts"], f["BT"]
            zt, zt3 = f["zt"], f["zt3"]
            nc.vector.scalar_tensor_tensor(
                out=zt[:], in0=zt[:], scalar=0.2, in1=zt[:],
                op0=ALU.mult, op1=ALU.max)
            nc.vector.tensor_tensor(
                out=zt3, in0=zt3,
                in1=att_sb[:, None, 0:dat_w].to_broadcast([P, BT, dat_w]),
                op=ALU.mult)
            lg = pool.tile([P, BT * heads], F32, tag="lg")
            nc.vector.reduce_sum(
                out=lg[:].rearrange("p (b h) -> p b h", h=heads),
                in_=zt[:].rearrange("p (b h c) -> p b h c",
                                    c=csub, h=heads),
                axis=mybir.AxisListType.X)
            mg = pool.tile([P, BT * mrow], BF16, tag="mg")
            mg3 = mg[:].rearrange("p (b m) -> p b m", m=mrow)
            nc.scalar.activation(
                mg3[:, :, dat_w:dat_w + heads],
                lg[:].rearrange("p (b h) -> p b h", h=heads), AF.Exp)
            f["lg"] = lg
            f["mg"] = mg
            f["mg3"] = mg3

        def backend_mg(f):
            """msg multiply (DVE)."""
            gi, sts, BT = f["gi"], f["sts"], f["BT"]
            mg3, xzs3 = f["mg3"], f["xzs3"]
            wv = mg3[:, :, dat_w:dat_w + heads, None] \
                .to_broadcast([P, BT, heads, csub])
            nc.vector.tensor_tensor(
                out=mg3[:, :, 0:dat_w].rearrange(
                    "p b (h c) -> p b h c", h=heads),
                in0=xzs3[:, :, src_lo:src_lo + dat_w].rearrange(
                    "p b (h c) -> p b h c", h=heads),
                in1=wv, op=ALU.mult)

        def backend_scatter(f):
            """per-supertile scatter matmul chains (PE)."""
            gi, sts, BT = f["gi"], f["sts"], f["BT"]
            mg, s4 = f["mg"], f["s4"]
            blist_full = f["blist"] + list(sts)
            out = []
            for stl, st in enumerate(sts):
                cols = [t for t, s in enumerate(blist_full) if s == st]
                agg = psum_agg.tile([P, mrow], F32, tag="agg",
                                    name=f"agg_l{layer}_st{st}")
                for i, t in enumerate(cols):
                    nc.tensor.matmul(
                        out=agg[:],
                        lhsT=s4[:, t * P:(t + 1) * P],
                        rhs=mg[:, t * mrow:(t + 1) * mrow],
                        start=(i == 0), stop=(i == len(cols) - 1))
                out.append((st, agg))
                if DEBUG_G0 and layer == 1 and gi == 0:
                    aggc = pool.tile([P, mrow], F32, tag=f"aggdbg{stl}")
                    nc.vector.tensor_copy(out=aggc[:], in_=agg[:])
                    g0dma(f"d_agg{stl}", aggc[:], [P, mrow], F32)
            if DEBUG_G0 and layer == 1 and gi == 0:
                g0dma("d_zt1", f["zt"][:], [P, BT * dat_w], BF16)
                g0dma("d_lg", f["lg"][:], [P, BT * heads], F32)
                g0dma("d_mg", mg[:], [P, BT * mrow], BF16)
            return out

        def epi_s0(e):
            """softmax divide: r0 on DVE, per-head scale on ACT."""
            for k, (st, agg) in enumerate(e):
                r0 = pool.tile([P, heads], F32, tag=f"r0_{k}")
                nc.vector.reciprocal(r0[:], agg[:, dat_w:dat_w + heads])
                ob = pool.tile([P, dat_w], F32, tag=f"ob_{k}")
                for h in range(heads):
                    nc.scalar.activation(
                        ob[:, h * csub:(h + 1) * csub],
                        agg[:, h * csub:(h + 1) * csub],
                        AF.Copy, scale=r0[:, h:h + 1])
                e[k] = (st, ob)

        def epi_s1(e):
            """bias add (DVE, SBUF-only)."""
            for k, (st, ob) in enumerate(e):
                ob2 = pool.tile([P, dat_w], F32, tag=f"ob2_{k}")
                nc.vector.tensor_tensor(
                    out=ob2[:], in0=ob[:], in1=bias_sb[:], op=ALU.add)
                e[k] = (st, ob2)

        def epi_s2(e):
            """ELU pieces on ACT: t2=exp(min(x,0)) via relu(-x), t3=relu(x)."""
            if layer != 1:
                return
            for k, (st, ob2) in enumerate(e):
                t1 = pool.tile([P, dat_w], F32, tag=f"t1_{k}")
                nc.scalar.activation(t1[:], ob2[:], AF.Relu, scale=-1.0)
                t2 = pool.tile([P, dat_w], F32, tag=f"t2_{k}")
                nc.scalar.activation(t2[:], t1[:], AF.Exp, scale=-1.0)
                t3 = pool.tile([P, dat_w], F32, tag=f"t3_{k}")
                nc.scalar.activation(t3[:], ob2[:], AF.Relu)
                e[k] = (st, (t2, t3))

        def epi_s3(e):
            """combine + store (+ interleaved dense-2 for layer 1)."""
            for k, (st, v) in enumerate(e):
                cnt = P if st < st_n - 1 else last
                if layer == 1:
                    t2, t3 = v
                    he = pool.tile([P, dat_w], BF16, tag=f"he_{k}")
                    nc.vector.scalar_tensor_tensor(
                        out=he[:], in0=t2[:], scalar=-1.0, in1=t3[:],
                        op0=ALU.add, op1=ALU.add)
                    if DEBUG_DUMP:
                        nc.sync.dma_start(
                            out=hdram[st * P:(st + 1) * P, :], in_=he[:])
                    if dense2 is not None:
                        dense2(st, he)
                else:
                    nc.sync.dma_start(
                        out=out_loc[st * P:st * P + cnt, :],
                        in_=v[:cnt, :])

        # skewed pipeline: frontend(i) | backend(i-SKEW) | epilogue(i-SKEW-1).
        # Each engine's queue only sees ops whose inputs were produced at
        # least one group earlier, so nothing blocks head-of-line.
        fs = {}     # frontend state per group
        es = {}     # scatter outputs per group
        ng = len(groups)
        sk = PIPE_SKEW
        for i in range(ng + sk + 1):
            gf = i if i < ng else None
            gb = i - sk if 0 <= i - sk < ng else None
            ge = i - sk - 1 if 0 <= i - sk - 1 < ng else None
            if gf is not None:
                fs[gf] = frontend_loads(gf, groups[gf])
            if sk == 1:
                if ge is not None:
                    epi_s0(es[ge])
                if gb is not None:
                    backend_a(fs[gb])
                if ge is not None:
                    epi_s1(es[ge])
                    epi_s2(es[ge])
                if gf is not None:
                    frontend_compute(fs[gf])
                if gb is not None:
                    backend_mg(fs[gb])
                    es[gb] = backend_scatter(fs[gb])
                if ge is not None:
                    epi_s3(es[ge])
            else:
                if gb is not None:
                    backend_a(fs[gb])
                    backend_mg(fs[gb])
                if ge is not None:
                    epi_s0(es[ge])
                if gf is not None:
                    frontend_compute(fs[gf])
                if ge is not None:
                    epi_s1(es[ge])
                    epi_s2(es[ge])
                if gb is not None:
                    es[gb] = backend_scatter(fs[gb])
                if ge is not None:
                    epi_s3(es[ge])

    for rep in range(reps):
        # ---- TC_A: sharded dense-1 (xl1 + xr1 for local nodes) ----
        with TileContext(nc) as tc:
            with tc.tile_pool(name="constA", bufs=1) as cpool, \
                 tc.tile_pool(name="dA", bufs=4) as dpool, \
                 tc.tile_pool(name="pdA", bufs=2, space="PSUM") as psum_d:
                w1_sb = cpool.tile([P, 2 * H1], BF16)
                nc.sync.dma_start(out=w1_sb[:], in_=W1[:])
                for t in range(st_n):
                    cols = P if t < st_n - 1 else last
                    xt = dpool.tile([P, P], BF16, tag="xt")
                    nc.sync.dma_start(out=xt[:, :cols],
                                      in_=xTo[:, t * P:t * P + cols])
                    psd = psum_d.tile([cols, 2 * H1], F32, tag="psd")
                    nc.tensor.matmul(out=psd[:], lhsT=xt[:, :cols],
                                     rhs=w1_sb[:], start=True, stop=True)
                    sb = dpool.tile([cols, 2 * H1], BF16, tag="sbd")
                    nc.scalar.copy(out=sb[:, 0:H1], in_=psd[:, 0:H1])
                    nc.vector.tensor_copy(out=sb[:, H1:2 * H1],
                                          in_=psd[:, H1:2 * H1])
                    nc.sync.dma_start(out=xl1_loc[t * P:t * P + cols, :],
                                      in_=sb[:, 0:H1])
                    nc.sync.dma_start(out=xr1[t * P:t * P + cols, :],
                                      in_=sb[:, H1:2 * H1])

        # ---- AllGather xl1_loc -> xl1 ----
        nc.gpsimd.collective_compute(
            "AllGather", ALU.bypass,
            replica_groups=[list(range(ncores))],
            ins=[xl1_loc[:]], outs=[xl1[:]],
        ).then_inc(cc_sem)
        nc.gpsimd.wait_ge(cc_sem, 2 * rep + 1)

        # ---- TC_B: L1 edges + interleaved dense-2 ----
        with TileContext(nc) as tc:
            with tc.tile_pool(name="const", bufs=1) as cpool, \
                 tc.tile_pool(name="work", bufs=2) as pool, \
                 tc.tile_pool(name="pagg", bufs=4, space="PSUM") as psum_agg, \
                 tc.tile_pool(name="pzb", bufs=2, space="PSUM") as psum_zb, \
                 tc.tile_pool(name="paux", bufs=2, space="PSUM") as psum_aux:
                nc.gpsimd.load_library(library_config.mlp)
                att1_sb = cpool.tile([P, H1], BF16)
                nc.sync.dma_start(out=att1_sb[:], in_=att1p[:])
                b1_sb = cpool.tile([P, H1], F32)
                nc.sync.dma_start(out=b1_sb[:], in_=b1p[:])
                colix_sb = cpool.tile([P, P], BF16)
                nc.sync.dma_start(out=colix_sb[:], in_=colix[:])
                rowix_sb = cpool.tile([P, 1], BF16)
                nc.sync.dma_start(out=rowix_sb[:], in_=rowixt[:])
                ones_sb = cpool.tile([1, P], BF16)
                nc.sync.dma_start(out=ones_sb[:], in_=ones1t[:])
                w2_sb = cpool.tile([P, 2 * OUT], BF16)
                nc.sync.dma_start(out=w2_sb[:], in_=W2[:])
                consts = {"colix": colix_sb, "rowix": rowix_sb,
                          "ones1": ones_sb}

                def dense2(st, he):
                    cols = P if st < st_n - 1 else last
                    xh = pool.tile([P, P], BF16, tag="xh")
                    nc.sync.dma_start_transpose(out=xh[:], in_=he[:])
                    psd2 = psum_aux.tile([P, 2 * OUT], F32, tag="psd2",
                                         name=f"psd2_{st}")
                    nc.tensor.matmul(out=psd2[:cols, :], lhsT=xh[:, :cols],
                                     rhs=w2_sb[:], start=True, stop=True)
                    sb2 = pool.tile([P, 2 * OUT], BF16, tag="sb2")
                    nc.scalar.copy(out=sb2[:cols, :], in_=psd2[:cols, :])
                    nc.sync.dma_start(out=t2loc[st * P:st * P + cols, :],
                                      in_=sb2[:cols, :])

                edge_layer((pool, psum_agg, psum_zb), consts,
                           xl1[0:H_SPLIT, :], xl1[H_SPLIT:n, :], xr1,
                           xl1_loc,
                           H1, HEADS, 0, 0, att1_sb, b1_sb, layer=1,
                           dense2=dense2)

        if DEBUG_DUMP and rep == 0:
            with TileContext(nc):
                for nm, (dst, src) in dbg_t.items():
                    nc.sync.dma_start(out=dst[:], in_=src[:])

        # ---- AllGather t2loc -> t2 ----
        nc.gpsimd.collective_compute(
            "AllGather", ALU.bypass,
            replica_groups=[list(range(ncores))],
            ins=[t2loc[:]], outs=[t2[:]],
        ).then_inc(cc_sem)
        nc.gpsimd.wait_ge(cc_sem, 2 * rep + 2)

        # ---- TC_C: edges layer 2 ----
        with TileContext(nc) as tc:
            with tc.tile_pool(name="const2", bufs=1) as cpool, \
                 tc.tile_pool(name="work2", bufs=2) as pool, \
                 tc.tile_pool(name="pagg2", bufs=4, space="PSUM") as psum_agg, \
                 tc.tile_pool(name="pzb2", bufs=2, space="PSUM") as psum_zb:
                nc.gpsimd.load_library(library_config.mlp)
                att2_sb = cpool.tile([P, OUT], BF16)
                nc.sync.dma_start(out=att2_sb[:], in_=att2p[:])
                b2_sb = cpool.tile([P, OUT], F32)
                nc.sync.dma_start(out=b2_sb[:], in_=b2r[:])
                colix_sb = cpool.tile([P, P], BF16)
                nc.sync.dma_start(out=colix_sb[:], in_=colix[:])
                rowix_sb = cpool.tile([P, 1], BF16)
                nc.sync.dma_start(out=rowix_sb[:], in_=rowixt[:])
                ones_sb = cpool.tile([1, P], BF16)
                nc.sync.dma_start(out=ones_sb[:], in_=ones1t[:])
                consts = {"colix": colix_sb, "rowix": rowix_sb,
                          "ones1": ones_sb}
                edge_layer((pool, psum_agg, psum_zb), consts,
                           t2[0:H_SPLIT, :], t2[H_SPLIT:n, :], t2loc,
                           t2loc,
                           OUT, 1, 0, OUT, att2_sb, b2_sb, layer=2)

    split_multi_waits(nc)
    mybir.codegen_inst_isa_subclasses(nc)
    return nc


# ---------------- entry point ----------------
def kernel(**inputs) -> np.ndarray:
    in_maps, meta = prep(inputs)
    nc = build_program(meta)
    res = run_bass_kernel_spmd(nc, in_maps, list(range(NCORES)))
    out = np.concatenate([res.results[c]["out_loc"] for c in range(NCORES)],
                         axis=0)
    return out.astype(np.float32)
